# revision 19
# baseline (speedup 1.0000x reference)
"""CLIP-style contrastive loss kernel for Trainium2 (8 NeuronCores, SPMD).

Math (reference simplification):
  v1n = vectors1 / ||row||;  v2n = vectors2 / ||row||
  l[i,j] = (v1n[i] . v2n[j]) * exp(t)
  loss   = (1/(2N)) * sum_i [ log(sum_j exp(l[i,j]) + EPS) - l[i,i] ]

Sharding: rows of vectors1 split across 8 cores (1024 rows each); vectors2
replicated (host-side, no collectives).  Both matrices are fed pre-transposed
[D, rows] in bf16 so the PE contracts over D on partitions.  vectors2 columns
are rotated per-core so each core's diagonal block lands at j' in [0, 1024),
making the diag-extraction program identical on all cores.

On-device pipeline per 1024-wide j-superchunk (jp):
  DMA v2t slice -> DVE square -> PE ones-matmul (col-sum over d) -> ACT
  ln -> ACT exp(-0.5 x) = rsqrt -> DVE scale (normalize v2 columns) ->
  PE main matmuls (raw v1 x v2n) -> ACT exp(scale=r1et[i]) with fused
  free-axis accumulation = per-row sum of exp.
Diagonal extracted from jp==0 psum (TT-mult with identity + reduce).
Finalize: log(rowsum+eps) - diag*r1et, partition-reduce via ones-matmul,
one scalar out per core; host sums 8 scalars / (2N).
"""

import sys

sys.path.insert(0, "/opt/trn_rl_repo")

from contextlib import ExitStack

import ml_dtypes
import numpy as np

import concourse.bass as bass
import concourse.tile as tile
from concourse import bacc, mybir
from concourse.bass_utils import run_bass_kernel_spmd
from concourse.masks import make_identity

P = 128
D = 512
N = 8192
NCORES = 8
R = N // NCORES          # 1024 rows of vectors1 per core
ND = D // P              # 4 d-tiles
NI = R // P              # 8 i-chunks per core
JW = 1024                # j-superchunk width
NJP = N // JW            # 8 j-superchunks
HW = 512                 # matmul free-dim (half of JW)
EPS = 0.001

F32 = mybir.dt.float32
BF16 = mybir.dt.bfloat16
AF = mybir.ActivationFunctionType
ALU = mybir.AluOpType

_CACHE = {}


def _build(loop_k=None):
    """Build the Bass program. loop_k wraps the whole body in an on-device
    For_i loop (benchmark builds only)."""
    nc = bacc.Bacc(
        "TRN2",
        target_bir_lowering=False,
        debug=False,
        enable_asserts=False,
        num_devices=NCORES,
    )
    v1t = nc.declare_dram_parameter("v1t", [D, R], BF16, isOutput=False)
    v2t = nc.declare_dram_parameter("v2t", [D, N], BF16, isOutput=False)
    tsc = nc.declare_dram_parameter("tsc", [1], F32, isOutput=False)
    out_d = nc.declare_dram_parameter("out", [1, 1], F32, isOutput=True)

    v1t3 = v1t.rearrange("(dt p) r -> p dt r", p=P)
    v2t3 = v2t.rearrange("(dt p) n -> p dt n", p=P)

    # Preload the one ACT table set containing BOTH exp and ln; otherwise the
    # auto-insert pass alternates exp_and_others <-> natural_log (55 table
    # loads, ~110us of ACT time).
    from concourse.hw_specs import get_activation_tables

    _tabs = list(get_activation_tables(nc.m.arch).items())
    _combined_id = next(
        i for i, (_, fns) in enumerate(_tabs)
        if AF.Exp in fns and AF.Ln in fns
    )

    with ExitStack() as ctx:
        tc = ctx.enter_context(tile.TileContext(nc))
        nc.scalar.add_instruction(
            mybir.InstLoadActFuncSet(
                name=nc.get_next_instruction_name(),
                ins=[],
                outs=[],
                act_func_set_id=_combined_id,
            )
        )
        singles = ctx.enter_context(tc.tile_pool(name="singles", bufs=1))
        v2pool = ctx.enter_context(tc.tile_pool(name="v2pool", bufs=3))
        v2npool = ctx.enter_context(tc.tile_pool(name="v2npool", bufs=2))
        work = ctx.enter_context(tc.tile_pool(name="work", bufs=3))
        exppool = ctx.enter_context(tc.tile_pool(name="exppool", bufs=2))
        psum_s = ctx.enter_context(tc.tile_pool(name="psum_s", bufs=2, space="PSUM"))
        psum_n = ctx.enter_context(tc.tile_pool(name="psum_n", bufs=2, space="PSUM"))
        psum_sm = ctx.enter_context(tc.tile_pool(name="psum_sm", bufs=2, space="PSUM"))

        # --- persistent constants --------------------------------------------
        t128 = singles.tile([P, 1], F32)
        nc.sync.dma_start(out=t128, in_=tsc[:].to_broadcast((P, 1)))
        ones_col = singles.tile([P, 1], BF16)
        nc.vector.memset(ones_col, 1.0)
        onesT = singles.tile([P, P], BF16)
        nc.vector.memset(onesT, 1.0)
        ident = singles.tile([P, P], F32)
        make_identity(nc, ident)
        ones_f32 = singles.tile([P, 1], F32)
        nc.vector.memset(ones_f32, 1.0)
        eps_t = singles.tile([P, 1], F32)
        nc.vector.memset(eps_t, EPS)
        rsums = singles.tile([P, NI, NJP], F32)
        r1et = singles.tile([P, NI], F32)
        qdiag = singles.tile([P, NI], F32)
        pers = singles.tile([P, NI], F32)

        def body():
            v1sb = singles.tile([P, ND, R], BF16, tag="v1sb")
            nc.sync.dma_start(out=v1sb, in_=v1t3)

            def phase_a():
                # r1et[i] = exp(t) / ||v1_i||
                vsq1 = singles.tile([P, ND, R], BF16, tag="vsq1")
                for dt_i in range(ND):
                    nc.vector.tensor_mul(vsq1[:, dt_i], v1sb[:, dt_i], v1sb[:, dt_i])
                for c in range(NI):
                    n1ps = psum_sm.tile([P, NI], F32, tag="sm")
                    for dt_i in range(ND):
                        nc.tensor.matmul(
                            n1ps[:, 0:1],
                            lhsT=vsq1[:, dt_i, c * P:(c + 1) * P],
                            rhs=ones_col,
                            start=(dt_i == 0),
                            stop=(dt_i == ND - 1),
                        )
                    lnt = work.tile([P, 1], F32, tag="lnt1")
                    nc.scalar.activation(lnt, n1ps[:, 0:1], AF.Ln)
                    # exp(-0.5*ln(|v1|^2) + t) = exp(t)/|v1|
                    nc.scalar.activation(
                        r1et[:, c:c + 1], lnt, AF.Exp, bias=t128[:, 0:1], scale=-0.5
                    )

            # --- phase B: stream j-superchunks -------------------------------
            def emit_norm(jp):
                """DMA + normalize the jp-th [D, JW] slice of v2t."""
                v2raw = v2pool.tile([P, ND, JW], BF16, tag="v2raw")
                nc.sync.dma_start(out=v2raw, in_=v2t3[:, :, jp * JW:(jp + 1) * JW])
                v2n = v2npool.tile([P, ND, JW], BF16, tag="v2n")
                for h in range(JW // HW):
                    hs = slice(h * HW, (h + 1) * HW)
                    nps = psum_n.tile([P, HW], F32, tag="nps")
                    for dt_i in range(ND):
                        vsq2 = work.tile([P, HW], BF16, tag="vsq2")
                        nc.vector.tensor_mul(
                            vsq2, v2raw[:, dt_i, hs], v2raw[:, dt_i, hs]
                        )
                        nc.tensor.matmul(
                            nps, lhsT=onesT, rhs=vsq2,
                            start=(dt_i == 0), stop=(dt_i == ND - 1),
                        )
                    lnm = work.tile([P, HW], F32, tag="lnm")
                    nc.scalar.activation(lnm, nps, AF.Ln)
                    r2b = work.tile([P, HW], BF16, tag="r2b")
                    nc.scalar.activation(r2b, lnm, AF.Exp, scale=-0.5)
                    for dt_i in range(ND):
                        nc.vector.tensor_mul(
                            v2n[:, dt_i, hs], v2raw[:, dt_i, hs], r2b
                        )
                return v2n

            phase_a()
            LOOKAHEAD = 1
            v2ns = {jp: emit_norm(jp) for jp in range(LOOKAHEAD)}
            for jp in range(NJP):
                if jp + LOOKAHEAD < NJP:
                    v2ns[jp + LOOKAHEAD] = emit_norm(jp + LOOKAHEAD)
                v2n = v2ns.pop(jp)
                for c in range(NI):
                    sps = psum_s.tile([P, JW], F32, tag="sps")
                    for dt_i in range(ND):
                        for h in range(JW // HW):
                            nc.tensor.matmul(
                                sps[:, h * HW:(h + 1) * HW],
                                lhsT=v1sb[:, dt_i, c * P:(c + 1) * P],
                                rhs=v2n[:, dt_i, h * HW:(h + 1) * HW],
                                start=(dt_i == 0),
                                stop=(dt_i == ND - 1),
                            )
                    if jp == 0:
                        scr = work.tile([P, P], F32, tag="diag_scr")
                        nc.vector.tensor_mul(scr, sps[:, c * P:(c + 1) * P], ident)
                        nc.vector.tensor_reduce(
                            qdiag[:, c:c + 1], scr,
                            axis=mybir.AxisListType.X, op=ALU.add,
                        )
                    ex = exppool.tile([P, JW], F32, tag="ex")
                    nc.scalar.activation(
                        ex, sps, AF.Exp,
                        scale=r1et[:, c:c + 1],
                        accum_out=rsums[:, c, jp:jp + 1],
                    )

            # --- finalize -----------------------------------------------------
            for c in range(NI):
                rs = work.tile([P, 1], F32, tag="rs")
                nc.vector.tensor_reduce(
                    rs, rsums[:, c], axis=mybir.AxisListType.X, op=ALU.add
                )
                lg = work.tile([P, 1], F32, tag="lg")
                nc.scalar.activation(lg, rs, AF.Ln, bias=eps_t[:, 0:1])
                qs = work.tile([P, 1], F32, tag="qs")
                nc.vector.tensor_mul(qs, qdiag[:, c:c + 1], r1et[:, c:c + 1])
                nc.vector.tensor_sub(pers[:, c:c + 1], lg, qs)
            fin = psum_sm.tile([P, NI], F32, tag="sm")
            nc.tensor.matmul(
                fin[0:1, :], lhsT=ones_f32, rhs=pers, start=True, stop=True
            )
            res = singles.tile([1, 1], F32, tag="res")
            nc.vector.tensor_reduce(
                res, fin[0:1, :], axis=mybir.AxisListType.X, op=ALU.add
            )
            nc.sync.dma_start(out=out_d[:], in_=res)

        if loop_k is None:
            body()
        else:
            with tc.For_i(0, loop_k, 1):
                body()

    nc.compile()
    return nc


def _get_nc():
    if "nc" not in _CACHE:
        _CACHE["nc"] = _build()
    return _CACHE["nc"]


def make_in_maps(vectors1, vectors2, t):
    v1 = np.asarray(vectors1, dtype=np.float32)
    v2 = np.asarray(vectors2, dtype=np.float32)
    tv = np.asarray(t, dtype=np.float32).reshape(1)
    v1t_full = np.ascontiguousarray(v1.T.astype(ml_dtypes.bfloat16))   # [D, N]
    v2t_full = np.ascontiguousarray(v2.T.astype(ml_dtypes.bfloat16))   # [D, N]
    in_maps = []
    for c in range(NCORES):
        v1t_c = np.ascontiguousarray(v1t_full[:, c * R:(c + 1) * R])
        # rotate columns so this core's diagonal block sits at j' in [0, R)
        v2t_c = np.ascontiguousarray(np.roll(v2t_full, -c * R, axis=1))
        in_maps.append({"v1t": v1t_c, "v2t": v2t_c, "tsc": tv})
    return in_maps


def kernel(vectors1, vectors2, t, **_unused):
    nc = _get_nc()
    in_maps = make_in_maps(vectors1, vectors2, t)
    results = run_bass_kernel_spmd(nc, in_maps, core_ids=list(range(NCORES))).results
    total = sum(float(r["out"][0, 0]) for r in results)
    return np.float32(total / N / 2.0)
